# revision 13
# baseline (speedup 1.0000x reference)
"""Trainium2 Bass kernel: per-batch-row stable partition (facts first, pads last).

For each batch row b: out[b] = sentout[b][order] where order lists positions
with nl_input[b] != 0 first (original order), then positions == 0.

Design notes (v3, gather form; v2 scatter form measured 94.6us):
  - The DMA subsystem is the wall.  Measured per-core model: 16 DMA engines,
    each min(~27 GB/s, pkt_size/145ns) -> ~430 GB/s aggregate for packets
    >= 4KB, linearly worse below 4KB.  A pure-streaming probe with 16KB
    descriptors also capped at ~430 GB/s, so bytes (not descriptors) are
    binding once every packet is >= 4KB.
  - f32 row scatter moves 33.6MB/core (16.8 in + 16.8 out) = ~80us at cap.
    Writing bf16 instead halves write bytes (rel-err ~1.7e-3, gate is 2e-2),
    but a bf16 *scatter* emits 2KB packets which run at half rate.  The fix
    is the gather form: read rows in OUTPUT order (4KB f32 packets, at cap),
    cast f32->bf16 on-chip, store contiguous multi-row bf16 descriptors
    (>= 4KB, at cap).  25.2MB/core at ~430 GB/s ~= 59us streaming.
  - Pure data parallel over B=16 on 8 cores (2 rows/core); kernel() takes
    full inputs, shards on host, gathers full output.
  - The gather index list (stable argsort of is_pad per row) is computed on
    host during input sharding and shipped as a 16KB int32 tensor per core,
    laid out so that each gather call's offset AP is one column.
  - Pipeline per block of output rows: G gpsimd indirect gathers (one per
    row-of-partition) -> cast (DVE/ACT alternating) -> HWDGE store on the
    sync/scalar queues (alternating).  Last blocks are smaller to shorten
    the drain tail.
"""

import numpy as np

import concourse.bass as bass
import concourse.mybir as mybir
import concourse.tile as tile
from concourse.bacc import Bacc
from concourse.bass_utils import run_bass_kernel_spmd

B, L, D = 16, 2048, 1024
NCORES = 8
BLOC = B // NCORES          # batch rows per core = 2
P = 128                     # SBUF partitions
RPC = BLOC * L              # rows per core = 4096

# Output-block plan: G = out rows per partition per block (block = P*G rows).
# bf16 store descriptor is G*D*2 bytes: G>=2 keeps it >= 4KB (at byte cap).
# Tail blocks are small so the final gather->cast->store drain is short.
BLOCKS = [4, 4, 4, 4, 4, 4, 4, 2, 2]
assert sum(BLOCKS) * P == RPC
NCOLS = sum(BLOCKS)         # 32 offset columns

_NC_CACHE = None


def _build_nc():
    f32 = mybir.dt.float32
    bf16 = mybir.dt.bfloat16
    i16 = mybir.dt.int16

    nc = Bacc()
    sent = nc.declare_dram_parameter("sent", [RPC, D], f32, isOutput=False)
    # int16 gather indices, SWDGE layout: for block k (wrapping position
    # i = j*128 + p, i.e. out row start_k + G*p + j), index i sits at
    # [i % 16, colbase_k + i // 16], replicated down all 128 partitions.
    idxg = nc.declare_dram_parameter("idxg", [P, RPC // 16], i16, isOutput=False)
    out = nc.declare_dram_parameter("out", [RPC, D], bf16, isOutput=True)

    with tile.TileContext(nc) as tc:
        with (
            tc.tile_pool(name="idx", bufs=1) as ipool,
            tc.tile_pool(name="f32", bufs=7) as fpool,
            tc.tile_pool(name="b16", bufs=7) as bpool,
        ):
            # gather offsets: tiny, head of the sync queue
            ot = ipool.tile([P, RPC // 16], i16)
            nc.sync.dma_start(ot[:], idxg[:])

            col16 = 0
            start = 0
            for k, G in enumerate(BLOCKS):
                rows = P * G
                ft = fpool.tile([P, G * D], f32, tag="f", name=f"f{k}")
                # one SWDGE ucode gather per block: 994ns fixed + 0.34ns/desc,
                # vs ~1us per 128-desc DMA_INDIRECT call
                nc.gpsimd.dma_gather(
                    out_ap=ft[:].rearrange("p (g d) -> p g d", g=G),
                    in_ap=sent[:],
                    idxs_ap=ot[:, col16 : col16 + rows // 16],
                    num_idxs=rows,
                    num_idxs_reg=rows,
                    elem_size=D,
                )
                bt = bpool.tile([P, G * D], bf16, tag="b", name=f"b{k}")
                if k % 2 == 0:
                    nc.vector.tensor_copy(bt[:], ft[:])
                else:
                    nc.scalar.activation(
                        bt[:], ft[:], mybir.ActivationFunctionType.Copy
                    )
                e = nc.sync if k % 2 == 0 else nc.scalar
                e.dma_start(
                    out[start : start + rows, :].rearrange(
                        "(p g) d -> p (g d)", p=P
                    ),
                    bt[:],
                )
                col16 += rows // 16
                start += rows
    nc.compile()
    return nc


def _get_nc():
    global _NC_CACHE
    if _NC_CACHE is None:
        _NC_CACHE = _build_nc()
    return _NC_CACHE


def _make_in_maps(sentout, nl_input):
    sent = np.ascontiguousarray(np.asarray(sentout, dtype=np.float32)).reshape(
        NCORES, RPC, D
    )
    # host side of the work split: the gather permutation (stable partition:
    # facts first, pads last, both in original order) in per-block layout
    nl = np.asarray(nl_input).reshape(NCORES, BLOC, L)
    is_pad = (nl == 0).astype(np.uint8)
    order = np.argsort(is_pad, axis=2, kind="stable").astype(np.int32)
    src = (order + (np.arange(BLOC, dtype=np.int32) * L)[None, :, None]).reshape(
        NCORES, RPC
    )
    # SWDGE gather index layout per block: position i = j*128 + p holds the
    # source of out row start_k + G*p + j; wrapped [i%16, i//16] over 16
    # partitions, replicated down all 128 partitions.
    idxg = np.empty((NCORES, 16, RPC // 16), dtype=np.int16)
    col16 = 0
    start = 0
    for G in BLOCKS:
        rows = P * G
        # blk[i] = src row of output row start + G*(i%128) + (i//128)
        blk = (
            src[:, start : start + rows]
            .reshape(NCORES, P, G)
            .transpose(0, 2, 1)
            .reshape(NCORES, rows)
        )
        idxg[:, :, col16 : col16 + rows // 16] = (
            blk.reshape(NCORES, rows // 16, 16).transpose(0, 2, 1)
        )
        col16 += rows // 16
        start += rows
    idxg = np.ascontiguousarray(
        np.broadcast_to(idxg[:, None, :, :], (NCORES, 8, 16, RPC // 16)).reshape(
            NCORES, P, RPC // 16
        )
    )
    return [{"sent": sent[c], "idxg": idxg[c]} for c in range(NCORES)]


def run_on_device(sentout, nl_input, **kwargs):
    """Run the Bass kernel; returns (full_output, BassKernelResults)."""
    nc = _get_nc()
    res = run_bass_kernel_spmd(
        nc, _make_in_maps(sentout, nl_input), core_ids=list(range(NCORES)), **kwargs
    )
    outs = [
        r["out"].astype(np.float32).reshape(BLOC, L, D) for r in res.results
    ]
    return np.concatenate(outs, axis=0), res


def kernel(sentout, nl_input):
    out, _ = run_on_device(sentout, nl_input)
    return out


# revision 14
# speedup vs baseline: 1.1482x; 1.1482x over previous
"""Trainium2 Bass kernel: per-batch-row stable partition (facts first, pads last).

For each batch row b: out[b] = sentout[b][order] where order lists positions
with nl_input[b] != 0 first (original order), then positions == 0.

Design notes (v3, gather form; v2 scatter form measured 94.6us):
  - The DMA subsystem is the wall.  Measured per-core model: 16 DMA engines,
    each min(~27 GB/s, pkt_size/145ns) -> ~430 GB/s aggregate for packets
    >= 4KB, linearly worse below 4KB.  A pure-streaming probe with 16KB
    descriptors also capped at ~430 GB/s, so bytes (not descriptors) are
    binding once every packet is >= 4KB.
  - f32 row scatter moves 33.6MB/core (16.8 in + 16.8 out) = ~80us at cap.
    Writing bf16 instead halves write bytes (rel-err ~1.7e-3, gate is 2e-2),
    but a bf16 *scatter* emits 2KB packets which run at half rate.  The fix
    is the gather form: read rows in OUTPUT order (4KB f32 packets, at cap),
    cast f32->bf16 on-chip, store contiguous multi-row bf16 descriptors
    (>= 4KB, at cap).  25.2MB/core at ~430 GB/s ~= 59us streaming.
  - Pure data parallel over B=16 on 8 cores (2 rows/core); kernel() takes
    full inputs, shards on host, gathers full output.
  - The gather index list (stable argsort of is_pad per row) is computed on
    host during input sharding and shipped as a 16KB int32 tensor per core,
    laid out so that each gather call's offset AP is one column.
  - Pipeline per block of output rows: G gpsimd indirect gathers (one per
    row-of-partition) -> cast (DVE/ACT alternating) -> HWDGE store on the
    sync/scalar queues (alternating).  Last blocks are smaller to shorten
    the drain tail.
"""

import numpy as np

import concourse.bass as bass
import concourse.mybir as mybir
import concourse.tile as tile
from concourse.bacc import Bacc
from concourse.bass_utils import run_bass_kernel_spmd

B, L, D = 16, 2048, 1024
NCORES = 8
BLOC = B // NCORES          # batch rows per core = 2
P = 128                     # SBUF partitions
RPC = BLOC * L              # rows per core = 4096

# Output-block plan: G = out rows per partition per block (block = P*G rows).
# bf16 store descriptor is G*D*2 bytes: G>=2 keeps it >= 4KB (at byte cap).
# Tail blocks are small so the final gather->cast->store drain is short.
BLOCKS = [4, 4, 4, 4, 4, 4, 4, 2, 2]
assert sum(BLOCKS) * P == RPC
NCOLS = sum(BLOCKS)         # 32 offset columns

_NC_CACHE = None


def _build_nc():
    f32 = mybir.dt.float32
    bf16 = mybir.dt.bfloat16
    i16 = mybir.dt.int16

    nc = Bacc()
    sent = nc.declare_dram_parameter("sent", [RPC, D], f32, isOutput=False)
    # int16 gather indices, SWDGE layout: for block k (wrapping position
    # i = j*128 + p, i.e. out row start_k + G*p + j), index i sits at
    # [i % 16, colbase_k + i // 16], replicated down all 128 partitions.
    idxg = nc.declare_dram_parameter("idxg", [P, RPC // 16], i16, isOutput=False)
    out = nc.declare_dram_parameter("out", [RPC, D], bf16, isOutput=True)

    with tile.TileContext(nc) as tc:
        with (
            tc.tile_pool(name="idx", bufs=1) as ipool,
            tc.tile_pool(name="f32", bufs=7) as fpool,
            tc.tile_pool(name="b16", bufs=7) as bpool,
        ):
            # gather offsets: tiny, head of the sync queue
            ot = ipool.tile([P, RPC // 16], i16)
            nc.sync.dma_start(ot[:], idxg[:])

            col16 = 0
            start = 0
            for k, G in enumerate(BLOCKS):
                rows = P * G
                ft = fpool.tile([P, G * D], f32, tag="f", name=f"f{k}")
                # one SWDGE ucode gather per block: 994ns fixed + 0.34ns/desc,
                # vs ~1us per 128-desc DMA_INDIRECT call
                nc.gpsimd.dma_gather(
                    out_ap=ft[:].rearrange("p (g d) -> p g d", g=G),
                    in_ap=sent[:],
                    idxs_ap=ot[:, col16 : col16 + rows // 16],
                    num_idxs=rows,
                    num_idxs_reg=rows,
                    elem_size=D,
                    # per-row 4KB packets: still at the DMA byte cap, and the
                    # per-packet arbiter then can't starve the store queues
                    # (128KB bundles monopolized all 16 engines)
                    single_packet=False,
                )
                bt = bpool.tile([P, G * D], bf16, tag="b", name=f"b{k}")
                if k % 2 == 0:
                    nc.vector.tensor_copy(bt[:], ft[:])
                else:
                    nc.scalar.activation(
                        bt[:], ft[:], mybir.ActivationFunctionType.Copy
                    )
                e = nc.sync if k % 2 == 0 else nc.scalar
                e.dma_start(
                    out[start : start + rows, :].rearrange(
                        "(p g) d -> p (g d)", p=P
                    ),
                    bt[:],
                )
                col16 += rows // 16
                start += rows
    nc.compile()
    return nc


def _get_nc():
    global _NC_CACHE
    if _NC_CACHE is None:
        _NC_CACHE = _build_nc()
    return _NC_CACHE


def _make_in_maps(sentout, nl_input):
    sent = np.ascontiguousarray(np.asarray(sentout, dtype=np.float32)).reshape(
        NCORES, RPC, D
    )
    # host side of the work split: the gather permutation (stable partition:
    # facts first, pads last, both in original order) in per-block layout
    nl = np.asarray(nl_input).reshape(NCORES, BLOC, L)
    is_pad = (nl == 0).astype(np.uint8)
    order = np.argsort(is_pad, axis=2, kind="stable").astype(np.int32)
    src = (order + (np.arange(BLOC, dtype=np.int32) * L)[None, :, None]).reshape(
        NCORES, RPC
    )
    # SWDGE gather index layout per block: position i = j*128 + p holds the
    # source of out row start_k + G*p + j; wrapped [i%16, i//16] over 16
    # partitions, replicated down all 128 partitions.
    idxg = np.empty((NCORES, 16, RPC // 16), dtype=np.int16)
    col16 = 0
    start = 0
    for G in BLOCKS:
        rows = P * G
        # blk[i] = src row of output row start + G*(i%128) + (i//128)
        blk = (
            src[:, start : start + rows]
            .reshape(NCORES, P, G)
            .transpose(0, 2, 1)
            .reshape(NCORES, rows)
        )
        idxg[:, :, col16 : col16 + rows // 16] = (
            blk.reshape(NCORES, rows // 16, 16).transpose(0, 2, 1)
        )
        col16 += rows // 16
        start += rows
    idxg = np.ascontiguousarray(
        np.broadcast_to(idxg[:, None, :, :], (NCORES, 8, 16, RPC // 16)).reshape(
            NCORES, P, RPC // 16
        )
    )
    return [{"sent": sent[c], "idxg": idxg[c]} for c in range(NCORES)]


def run_on_device(sentout, nl_input, **kwargs):
    """Run the Bass kernel; returns (full_output, BassKernelResults)."""
    nc = _get_nc()
    res = run_bass_kernel_spmd(
        nc, _make_in_maps(sentout, nl_input), core_ids=list(range(NCORES)), **kwargs
    )
    outs = [
        r["out"].astype(np.float32).reshape(BLOC, L, D) for r in res.results
    ]
    return np.concatenate(outs, axis=0), res


def kernel(sentout, nl_input):
    out, _ = run_on_device(sentout, nl_input)
    return out


# revision 19
# speedup vs baseline: 1.2855x; 1.1196x over previous
"""Trainium2 Bass kernel: per-batch-row stable partition (facts first, pads last).

For each batch row b: out[b] = sentout[b][order] where order lists positions
with nl_input[b] != 0 first (original order), then positions == 0.

Design notes (v3, gather form; v2 scatter form measured 94.6us):
  - The DMA subsystem is the wall.  Measured per-core model: 16 DMA engines,
    each min(~27 GB/s, pkt_size/145ns) -> ~430 GB/s aggregate for packets
    >= 4KB, linearly worse below 4KB.  A pure-streaming probe with 16KB
    descriptors also capped at ~430 GB/s, so bytes (not descriptors) are
    binding once every packet is >= 4KB.
  - f32 row scatter moves 33.6MB/core (16.8 in + 16.8 out) = ~80us at cap.
    Writing bf16 instead halves write bytes (rel-err ~1.7e-3, gate is 2e-2),
    but a bf16 *scatter* emits 2KB packets which run at half rate.  The fix
    is the gather form: read rows in OUTPUT order (4KB f32 packets, at cap),
    cast f32->bf16 on-chip, store contiguous multi-row bf16 descriptors
    (>= 4KB, at cap).  25.2MB/core at ~430 GB/s ~= 59us streaming.
  - Pure data parallel over B=16 on 8 cores (2 rows/core); kernel() takes
    full inputs, shards on host, gathers full output.
  - The gather index list (stable argsort of is_pad per row) is computed on
    host during input sharding and shipped as a 16KB int32 tensor per core,
    laid out so that each gather call's offset AP is one column.
  - Pipeline per block of output rows: G gpsimd indirect gathers (one per
    row-of-partition) -> cast (DVE/ACT alternating) -> HWDGE store on the
    sync/scalar queues (alternating).  Last blocks are smaller to shorten
    the drain tail.
"""

import numpy as np

import concourse.bass as bass
import concourse.mybir as mybir
import concourse.tile as tile
from concourse.bacc import Bacc
from concourse.bass_utils import run_bass_kernel_spmd

B, L, D = 16, 2048, 1024
NCORES = 8
BLOC = B // NCORES          # batch rows per core = 2
P = 128                     # SBUF partitions
RPC = BLOC * L              # rows per core = 4096

# Output-block plan: G = out rows per partition per block (block = P*G rows).
# bf16 store descriptor is G*D*2 bytes: G>=2 keeps it >= 4KB (at byte cap).
# Small HEAD blocks so the first stores issue early (the pre-store phase is
# read-only and feed-paced, leaving engine capacity idle); small TAIL blocks
# so the final gather->cast->store drain is short.
BLOCKS = [2, 2, 2, 2, 4, 4, 4, 4, 4, 2, 2]
assert sum(BLOCKS) * P == RPC
NCOLS = sum(BLOCKS)         # 32 offset columns

_NC_CACHE = None


def _build_nc():
    f32 = mybir.dt.float32
    bf16 = mybir.dt.bfloat16
    i32 = mybir.dt.int32

    nc = Bacc()
    sent = nc.declare_dram_parameter("sent", [RPC, D], f32, isOutput=False)
    # ordg[p, colbase_k + j] = source row of output row  start_k + G_k*p + j
    ordg = nc.declare_dram_parameter("ordg", [P, NCOLS], i32, isOutput=False)
    out = nc.declare_dram_parameter("out", [RPC, D], bf16, isOutput=True)

    with tile.TileContext(nc) as tc:
        with (
            tc.tile_pool(name="idx", bufs=1) as ipool,
            tc.tile_pool(name="f32", bufs=7) as fpool,
            tc.tile_pool(name="b16", bufs=7) as bpool,
        ):
            # gather offsets: tiny, head of the sync queue
            ot = ipool.tile([P, NCOLS], i32)
            nc.sync.dma_start(ot[:], ordg[:])

            col = 0
            start = 0
            for k, G in enumerate(BLOCKS):
                rows = P * G
                ft = fpool.tile([P, G * D], f32, tag="f", name=f"f{k}")
                # per-column DMA_INDIRECTs (~1us fixed + ~0.7ns/desc each).
                # A multi-column offset AP hard-crashes the exec unit
                # (NRT_EXEC_UNIT_UNRECOVERABLE); dma_gather's ucode both
                # starts ~6us later and generates descriptors ~13x slower.
                for j in range(G):
                    nc.gpsimd.indirect_dma_start(
                        out=ft[:, j * D : (j + 1) * D],
                        out_offset=None,
                        in_=sent[:],
                        in_offset=bass.IndirectOffsetOnAxis(
                            ap=ot[:, col + j : col + j + 1], axis=0
                        ),
                    )
                bt = bpool.tile([P, G * D], bf16, tag="b", name=f"b{k}")
                if k % 2 == 0:
                    nc.vector.tensor_copy(bt[:], ft[:])
                else:
                    nc.scalar.activation(
                        bt[:], ft[:], mybir.ActivationFunctionType.Copy
                    )
                e = nc.sync if k % 2 == 0 else nc.scalar
                e.dma_start(
                    out[start : start + rows, :].rearrange(
                        "(p g) d -> p (g d)", p=P
                    ),
                    bt[:],
                )
                col += G
                start += rows
    nc.compile()
    return nc


def _get_nc():
    global _NC_CACHE
    if _NC_CACHE is None:
        _NC_CACHE = _build_nc()
    return _NC_CACHE


def _make_in_maps(sentout, nl_input):
    sent = np.ascontiguousarray(np.asarray(sentout, dtype=np.float32)).reshape(
        NCORES, RPC, D
    )
    # host side of the work split: the gather permutation (stable partition:
    # facts first, pads last, both in original order) in per-block layout
    nl = np.asarray(nl_input).reshape(NCORES, BLOC, L)
    is_pad = (nl == 0).astype(np.uint8)
    order = np.argsort(is_pad, axis=2, kind="stable").astype(np.int32)
    src = (order + (np.arange(BLOC, dtype=np.int32) * L)[None, :, None]).reshape(
        NCORES, RPC
    )
    # per-block column layout: ordg[p, colbase_k + j] = src of out row
    # start_k + G_k*p + j  (partition p's j-th row of block k, so each
    # store descriptor covers G_k contiguous output rows)
    ordg = np.empty((NCORES, P, NCOLS), dtype=np.int32)
    col = 0
    start = 0
    for G in BLOCKS:
        rows = P * G
        blk = src[:, start : start + rows].reshape(NCORES, P, G)
        ordg[:, :, col : col + G] = blk
        col += G
        start += rows
    ordg = np.ascontiguousarray(ordg)
    return [{"sent": sent[c], "ordg": ordg[c]} for c in range(NCORES)]


def run_on_device(sentout, nl_input, **kwargs):
    """Run the Bass kernel; returns (full_output, BassKernelResults)."""
    nc = _get_nc()
    res = run_bass_kernel_spmd(
        nc, _make_in_maps(sentout, nl_input), core_ids=list(range(NCORES)), **kwargs
    )
    outs = [
        r["out"].astype(np.float32).reshape(BLOC, L, D) for r in res.results
    ]
    return np.concatenate(outs, axis=0), res


def kernel(sentout, nl_input):
    out, _ = run_on_device(sentout, nl_input)
    return out
